# revision 6
# baseline (speedup 1.0000x reference)
"""MultiHeadAttention Trainium2 kernel (8 NeuronCores, SPMD).

Reference computation (B=4, T=1024, D=768, H=12, Dh=64):
    q = x @ Wq.T ; k = x @ Wk.T ; v = x @ Wv.T       (per-head reshape)
    attn = softmax((q @ k.T) / 8)
    out = (attn @ v) @ Wo.T + bo

Sharding: 8 cores = 4 batches x 2 head-halves (6 heads each). Each core
emits two [1024, 768] partials (head-chunks m0+m1 and m2) of the output
projection; the host sums the four partials per batch and adds the bias.

All device data is bf16 (host-converted, fp32 PSUM accumulation), which
runs matmuls at 1 cycle/row for any moving-dim size and halves DMA bytes.

Per-core dataflow:
    qT,kT = (W x)  in [dh(384), t] layout (per m-chunk of 128 = 2 heads)
    v     = (x Wv) in [t, 6*(64+1)] tiles; col 64 of each head block is
            ones so the ctx matmul also emits softmax denominators
    S.T psum [kt(4x128), q(256)] = kT_head.T @ qT_head  (K=64, N=256)
    expS  = exp(S.T) bf16 via ScalarE reading PSUM; quarter-width q
            tiles so the last head's attention staggers per 2 q-chunks
    ctx[q, 65] psum += expS_j.T @ [v_j | 1]       (K=kt chunks, N=65)
        col 64 = denominator; normalize with DVE reciprocal +
        per-partition tensor_scalar_mul (denominator is per-q = per-row)
    ctxT: head-pairs 0,1 via DMA-XBAR sbuf transpose (bf16), pair 2 via
        PE transpose (latency-critical tail)
    out1[q, :] = ctxT_0.T @ Wo_0 + ctxT_1.T @ Wo_1   (psum accum, mid)
    out2[q, :] = ctxT_2.T @ Wo_2                     (tail, no add)
"""

import numpy as np
import ml_dtypes

import concourse.mybir as mybir
from concourse import bacc
from concourse.tile import TileContext
from concourse.bass_utils import run_bass_kernel_spmd

FP = mybir.dt.float32
BF = mybir.dt.bfloat16
AF = mybir.ActivationFunctionType
BF_NP = ml_dtypes.bfloat16

B, T, D = 4, 1024, 768
H, DH = 12, 64
NCORES = 8
HPC = 6           # heads per core
DPC = HPC * DH    # 384 head-dims per core
KC = D // 128     # 6 contraction chunks of d_in
MC = DPC // 128   # 3 chunks of per-core head dims (2 heads each)
QC = T // 128     # 8 query chunks
TT = T // 128     # 8 key chunks


def emit_mha(tc, xt, wk, wq, wv, wo, ident, o1d, o2d, ctx):
    nc = tc.nc

    singles = ctx.enter_context(tc.tile_pool(name="singles", bufs=1))
    # scores psum: [128,1024] fp32 = 2 banks each; 2 bufs = 4 banks
    sps = ctx.enter_context(tc.tile_pool(name="sps", bufs=2, space="PSUM"))
    # shared work psum (qk/v/ctx/transpose/out): 4 bufs x 1 bank = 4 banks
    wps = ctx.enter_context(tc.tile_pool(name="wps", bufs=4, space="PSUM"))
    expp = ctx.enter_context(tc.tile_pool(name="expp", bufs=34))
    osbp = ctx.enter_context(tc.tile_pool(name="osbp", bufs=4))

    # ---------------- SBUF singles ----------------
    xT_sb = singles.tile([128, KC, T], BF, name="xT_sb", tag="xT_sb")
    wk_sb = singles.tile([128, MC, 768], BF, name="wk_sb", tag="wk_sb")
    wq_sb = singles.tile([128, MC, 768], BF, name="wq_sb", tag="wq_sb")
    wv_sb = singles.tile([128, KC, DPC], BF, name="wv_sb", tag="wv_sb")
    wo_sb = singles.tile([128, MC, 768], BF, name="wo_sb", tag="wo_sb")
    id_sb = singles.tile([128, 128], BF, name="id_sb", tag="id_sb")
    kT_sb = singles.tile([128, MC, T], BF, name="kT_sb", tag="kT_sb")
    qT_sb = singles.tile([128, MC, T], BF, name="qT_sb", tag="qT_sb")
    ctxn_sb = singles.tile([128, QC, DPC], BF, name="ctxn_sb", tag="ctxn_sb")
    ctxT_sb = singles.tile([128, MC, T], BF, name="ctxT_sb", tag="ctxT_sb")
    rcp_sb = singles.tile([128, HPC * QC], FP, name="rcp_sb", tag="rcp_sb")
    v_sb = []
    for j in range(TT):
        vt = singles.tile([128, HPC, DH + 1], BF, name=f"v_sb{j}", tag=f"v_sb{j}")
        v_sb.append(vt)

    # ones columns for the fused softmax denominators (Pool engine, SBUF)
    for j in range(TT):
        nc.gpsimd.memset(v_sb[j][:, :, DH : DH + 1], 1.0)

    # ---------------- input DMAs (SP/HWDGE queue) ----------------
    xtr = xt.rearrange("p (c t) -> p c t", c=KC)
    nc.sync.dma_start(out=wk_sb[:, 0, :], in_=wk[:, 0:768])
    nc.sync.dma_start(out=xT_sb[:, 0, 0:512], in_=xtr[:, 0, 0:512])
    nc.sync.dma_start(out=wq_sb[:, 0, :], in_=wq[:, 0:768])
    for c in range(1, KC):
        nc.sync.dma_start(out=xT_sb[:, c, 0:512], in_=xtr[:, c, 0:512])
    for c in range(KC):
        nc.sync.dma_start(out=xT_sb[:, c, 512:1024], in_=xtr[:, c, 512:1024])
    nc.sync.dma_start(out=wk_sb[:, 1:3, :], in_=wk[:, 768:2304])
    nc.sync.dma_start(out=wq_sb[:, 1:3, :], in_=wq[:, 768:2304])
    nc.sync.dma_start(out=wv_sb, in_=wv.rearrange("p (c n) -> p c n", c=KC))
    nc.sync.dma_start(out=wo_sb, in_=wo.rearrange("p (m d) -> p m d", m=MC))
    nc.sync.dma_start(out=id_sb, in_=ident)

    # ---------------- atoms ----------------
    expS = {}

    def qk_half(m, n, w_sb, dst, act_copy):
        ps = wps.tile([128, 512], FP, name="ps_qk", tag="w")
        for c in range(KC):
            nc.tensor.matmul(
                ps,
                lhsT=w_sb[:, m, c * 128 : (c + 1) * 128],
                rhs=xT_sb[:, c, n * 512 : (n + 1) * 512],
                start=(c == 0),
                stop=(c == KC - 1),
            )
        if act_copy:
            nc.scalar.copy(dst[:, m, n * 512 : (n + 1) * 512], ps)
        else:
            nc.vector.tensor_copy(dst[:, m, n * 512 : (n + 1) * 512], ps)

    def score(h, qq, jq):
        # S.T for key chunks j = 4jq..4jq+3, query quarter qq, one head
        m, po = h // 2, 64 * (h % 2)
        ps = sps.tile([128, 1024], FP, name="ps_s", tag="s")
        for r in range(4):
            j = 4 * jq + r
            nc.tensor.matmul(
                ps[:, r * 256 : (r + 1) * 256],
                lhsT=kT_sb[po : po + 64, m, j * 128 : (j + 1) * 128],
                rhs=qT_sb[po : po + 64, m, qq * 256 : (qq + 1) * 256],
                start=True,
                stop=True,
            )
        ex = expp.tile([128, 1024], BF, name="ex", tag="ex")
        nc.scalar.activation(ex, ps, AF.Exp)
        expS[(h, qq, jq)] = ex

    def v_mt(mt):
        ps = wps.tile([128, DPC], FP, name="ps_v", tag="w")
        for c in range(KC):
            nc.tensor.matmul(
                ps,
                lhsT=xT_sb[:, c, mt * 128 : (mt + 1) * 128],
                rhs=wv_sb[:, c, :],
                start=(c == 0),
                stop=(c == KC - 1),
            )
        nc.vector.tensor_copy(v_sb[mt][:, :, 0:DH], ps)

    def ctx_pair(pair, qc):
        # ctx[q, dh|denom] for heads 2p,2p+1 in one psum tile [128, 130]
        pc = wps.tile([128, 130], FP, name="pc", tag="w")
        for hi in range(2):
            h = 2 * pair + hi
            col = hi * 65
            for j in range(TT):
                ex = expS[(h, qc // 2, j // 4)]
                off = (j % 4) * 256 + (qc % 2) * 128
                nc.tensor.matmul(
                    pc[:, col : col + 65],
                    lhsT=ex[:, off : off + 128],
                    rhs=v_sb[j][:, h, :],
                    start=(j == 0),
                    stop=(j == TT - 1),
                )
        for hi in range(2):
            h = 2 * pair + hi
            k = h * QC + qc
            nc.vector.reciprocal(
                rcp_sb[:, k : k + 1], pc[:, hi * 65 + 64 : hi * 65 + 65]
            )
            nc.vector.tensor_scalar_mul(
                ctxn_sb[:, qc, h * 64 : (h + 1) * 64],
                pc[:, hi * 65 : hi * 65 + 64],
                rcp_sb[:, k : k + 1],
            )

    def tpose_dma(pair, qc):
        # pairs 0/1: DMA-XBAR sbuf->sbuf bf16 transpose, off the PE/DVE path
        nc.sync.dma_start_transpose(
            out=ctxT_sb[:, pair, qc * 128 : (qc + 1) * 128],
            in_=ctxn_sb[:, qc, pair * 128 : (pair + 1) * 128],
        )

    def tpose_pe(pair, qc):
        tp = wps.tile([128, 128], BF, name="tp", tag="w")
        nc.tensor.matmul(
            tp,
            lhsT=ctxn_sb[:, qc, pair * 128 : (pair + 1) * 128],
            rhs=id_sb,
            is_transpose=True,
        )
        nc.vector.tensor_copy(ctxT_sb[:, pair, qc * 128 : (qc + 1) * 128], tp)

    def out01(qc):
        osb = osbp.tile([128, D], FP, name="osb1", tag="osb1")
        for n2 in range(2):
            ps = wps.tile([128, 384], FP, name="ps_o", tag="w")
            for m in range(2):
                nc.tensor.matmul(
                    ps,
                    lhsT=ctxT_sb[:, m, qc * 128 : (qc + 1) * 128],
                    rhs=wo_sb[:, m, n2 * 384 : (n2 + 1) * 384],
                    start=(m == 0),
                    stop=(m == 1),
                )
            nc.vector.tensor_copy(osb[:, n2 * 384 : (n2 + 1) * 384], ps)
        nc.sync.dma_start(out=o1d[qc * 128 : (qc + 1) * 128, :], in_=osb)

    def out2(qc):
        osb = osbp.tile([128, D], FP, name="osb2", tag="osb2")
        for n2 in range(2):
            ps = wps.tile([128, 384], FP, name="ps_o2", tag="w")
            nc.tensor.matmul(
                ps,
                lhsT=ctxT_sb[:, 2, qc * 128 : (qc + 1) * 128],
                rhs=wo_sb[:, 2, n2 * 384 : (n2 + 1) * 384],
                start=True,
                stop=True,
            )
            nc.vector.tensor_copy(osb[:, n2 * 384 : (n2 + 1) * 384], ps)
        nc.sync.dma_start(out=o2d[qc * 128 : (qc + 1) * 128, :], in_=osb)

    # ---------------- schedule ----------------
    # qk m0 chase: k/q n0 first (ScalarE copies) so head-0 scores start
    # while the n1 token-halves are still in flight.
    qk_half(0, 0, wk_sb, kT_sb, act_copy=True)
    qk_half(0, 0, wq_sb, qT_sb, act_copy=True)
    score(0, 0, 0)
    score(0, 1, 0)
    qk_half(0, 1, wk_sb, kT_sb, act_copy=False)
    score(0, 0, 1)
    score(0, 1, 1)
    qk_half(0, 1, wq_sb, qT_sb, act_copy=False)
    for qq in (2, 3):
        for jq in range(2):
            score(0, qq, jq)
    for qq in range(4):
        for jq in range(2):
            score(1, qq, jq)
    for n in range(2):
        qk_half(1, n, wk_sb, kT_sb, act_copy=False)
        qk_half(1, n, wq_sb, qT_sb, act_copy=False)
    for mt in range(TT):
        v_mt(mt)
    for qc in range(QC):
        ctx_pair(0, qc)
        tpose_dma(0, qc)
    for qq in range(4):
        for jq in range(2):
            score(2, qq, jq)
    for n in range(2):
        qk_half(2, n, wk_sb, kT_sb, act_copy=False)
        qk_half(2, n, wq_sb, qT_sb, act_copy=False)
    for qq in range(4):
        for jq in range(2):
            score(3, qq, jq)
    for qc in range(QC):
        ctx_pair(1, qc)
        tpose_dma(1, qc)
    for qq in range(4):
        for jq in range(2):
            score(4, qq, jq)
    for qc in range(QC):
        out01(qc)
    # head-5 scores up front so the ScalarE exp chain never stalls; only
    # the ctx/transpose/out2 tail staggers per query quarter (2 q-chunks)
    for qq in range(4):
        for jq in range(2):
            score(5, qq, jq)
    for g in range(4):
        for qc in (2 * g, 2 * g + 1):
            ctx_pair(2, qc)
        for qc in (2 * g, 2 * g + 1):
            tpose_pe(2, qc)
        for qc in (2 * g, 2 * g + 1):
            out2(qc)


_PROGRAM = None


def build_program():
    global _PROGRAM
    if _PROGRAM is not None:
        return _PROGRAM
    nc = bacc.Bacc("TRN2", target_bir_lowering=False, debug=False, num_devices=NCORES)
    xt = nc.dram_tensor("xt", (128, KC * T), BF, kind="ExternalInput").ap()
    wk = nc.dram_tensor("wk", (128, MC * 768), BF, kind="ExternalInput").ap()
    wq = nc.dram_tensor("wq", (128, MC * 768), BF, kind="ExternalInput").ap()
    wv = nc.dram_tensor("wv", (128, KC * DPC), BF, kind="ExternalInput").ap()
    wo = nc.dram_tensor("wo", (128, MC * 768), BF, kind="ExternalInput").ap()
    ident = nc.dram_tensor("ident", (128, 128), BF, kind="ExternalInput").ap()
    out1 = nc.dram_tensor("out1", (T, D), FP, kind="ExternalOutput").ap()
    out2 = nc.dram_tensor("out2", (T, D), FP, kind="ExternalOutput").ap()
    from contextlib import ExitStack

    with TileContext(nc) as tc, ExitStack() as st:
        emit_mha(tc, xt, wk, wq, wv, wo, ident, out1, out2, st)
    nc.compile()
    _PROGRAM = nc
    return nc


def _pack_kq(w):
    # [768 d_in, 384 dout] -> [128 p, (m, c, 128)] with d_in = c*128+p
    return np.ascontiguousarray(
        w.reshape(KC, 128, MC, 128).transpose(1, 2, 0, 3).reshape(128, MC * 768)
    ).astype(BF_NP)


def make_in_maps(x, Wq, Wk, Wv, Wo):
    x = np.asarray(x, dtype=np.float32)
    ident = np.eye(128, dtype=np.float32).astype(BF_NP)
    in_maps = []
    xTs = []
    for b in range(B):
        xb = x[b].T  # [768, 1024]
        xTs.append(
            np.ascontiguousarray(
                xb.reshape(KC, 128, T).transpose(1, 0, 2).reshape(128, KC * T)
            ).astype(BF_NP)
        )
    for core in range(NCORES):
        b, hh = core // 2, core % 2
        sl = slice(hh * DPC, (hh + 1) * DPC)
        wvT = np.asarray(Wv)[sl].T.astype(np.float32)  # [768, 384]
        woT = np.asarray(Wo)[:, sl].T.astype(np.float32)  # [384, 768]
        in_maps.append(
            {
                "xt": xTs[b],
                "wq": _pack_kq((np.asarray(Wq)[sl] * 0.125).T.astype(np.float32)),
                "wk": _pack_kq(np.asarray(Wk)[sl].T.astype(np.float32)),
                "wv": np.ascontiguousarray(
                    wvT.reshape(KC, 128, DPC).transpose(1, 0, 2).reshape(128, KC * DPC)
                ).astype(BF_NP),
                "wo": np.ascontiguousarray(
                    woT.reshape(MC, 128, 768).transpose(1, 0, 2).reshape(128, MC * 768)
                ).astype(BF_NP),
                "ident": ident,
            }
        )
    return in_maps


def kernel(x, Wq, Wk, Wv, Wo, bo):
    nc = build_program()
    in_maps = make_in_maps(x, Wq, Wk, Wv, Wo)
    res = run_bass_kernel_spmd(nc, in_maps, core_ids=list(range(NCORES)))
    bo = np.asarray(bo, dtype=np.float32)
    out = np.empty((B, T, D), dtype=np.float32)
    for b in range(B):
        out[b] = (
            res.results[2 * b]["out1"]
            + res.results[2 * b]["out2"]
            + res.results[2 * b + 1]["out1"]
            + res.results[2 * b + 1]["out2"]
            + bo
        )
    return out


# revision 8
# speedup vs baseline: 1.0265x; 1.0265x over previous
"""MultiHeadAttention Trainium2 kernel (8 NeuronCores, SPMD).

Reference computation (B=4, T=1024, D=768, H=12, Dh=64):
    q = x @ Wq.T ; k = x @ Wk.T ; v = x @ Wv.T       (per-head reshape)
    attn = softmax((q @ k.T) / 8)
    out = (attn @ v) @ Wo.T + bo

Sharding: 8 cores = 4 batches x 2 head-halves (6 heads each). Each core
emits two [1024, 768] partials (head-chunks m0+m1 and m2) of the output
projection; the host sums the four partials per batch and adds the bias.

All device data is bf16 (host-converted, fp32 PSUM accumulation), which
runs matmuls at 1 cycle/row for any moving-dim size and halves DMA bytes.

Per-core dataflow:
    qT,kT = (W x)  in [dh(384), t] layout (per m-chunk of 128 = 2 heads)
    v     = (x Wv) in [t, 6*(64+1)] tiles; col 64 of each head block is
            ones so the ctx matmul also emits softmax denominators
    S.T psum [kt(4x128), q(256)] = kT_head.T @ qT_head  (K=64, N=256)
    expS  = exp(S.T) bf16 via ScalarE reading PSUM; quarter-width q
            tiles so the last head's attention staggers per 2 q-chunks
    ctx[q, 65] psum += expS_j.T @ [v_j | 1]       (K=kt chunks, N=65)
        col 64 = denominator; normalize with DVE reciprocal +
        per-partition tensor_scalar_mul (denominator is per-q = per-row)
    ctxT: head-pairs 0,1 via DMA-XBAR sbuf transpose (bf16), pair 2 via
        PE transpose (latency-critical tail)
    out1[q, :] = ctxT_0.T @ Wo_0 + ctxT_1.T @ Wo_1   (psum accum, mid)
    out2[q, :] = ctxT_2.T @ Wo_2                     (tail, no add)
"""

import numpy as np
import ml_dtypes

import concourse.mybir as mybir
from concourse import bacc
from concourse.tile import TileContext
from concourse.bass_utils import run_bass_kernel_spmd

FP = mybir.dt.float32
BF = mybir.dt.bfloat16
AF = mybir.ActivationFunctionType
BF_NP = ml_dtypes.bfloat16

B, T, D = 4, 1024, 768
H, DH = 12, 64
NCORES = 8
HPC = 6           # heads per core
DPC = HPC * DH    # 384 head-dims per core
KC = D // 128     # 6 contraction chunks of d_in
MC = DPC // 128   # 3 chunks of per-core head dims (2 heads each)
QC = T // 128     # 8 query chunks
TT = T // 128     # 8 key chunks


def emit_mha(tc, xt, wk, wq, wv, wo, ident, o1d, ctx):
    nc = tc.nc

    singles = ctx.enter_context(tc.tile_pool(name="singles", bufs=1))
    # scores psum: [128,1024] fp32 = 2 banks each; 2 bufs = 4 banks
    sps = ctx.enter_context(tc.tile_pool(name="sps", bufs=2, space="PSUM"))
    # shared work psum (qk/v/ctx/transpose/out): 4 bufs x 1 bank = 4 banks
    wps = ctx.enter_context(tc.tile_pool(name="wps", bufs=4, space="PSUM"))
    expp = ctx.enter_context(tc.tile_pool(name="expp", bufs=34))
    osbp = ctx.enter_context(tc.tile_pool(name="osbp", bufs=4))

    # ---------------- SBUF singles ----------------
    xT_sb = singles.tile([128, KC, T], BF, name="xT_sb", tag="xT_sb")
    wk_sb = singles.tile([128, MC, 768], BF, name="wk_sb", tag="wk_sb")
    wq_sb = singles.tile([128, MC, 768], BF, name="wq_sb", tag="wq_sb")
    wv_sb = singles.tile([128, KC, DPC], BF, name="wv_sb", tag="wv_sb")
    wo_sb = singles.tile([128, MC, 768], BF, name="wo_sb", tag="wo_sb")
    id_sb = singles.tile([128, 128], BF, name="id_sb", tag="id_sb")
    kT_sb = singles.tile([128, MC, T], BF, name="kT_sb", tag="kT_sb")
    qT_sb = singles.tile([128, MC, T], BF, name="qT_sb", tag="qT_sb")
    ctxn_sb = singles.tile([128, QC, DPC], BF, name="ctxn_sb", tag="ctxn_sb")
    ctxT_sb = singles.tile([128, MC, T], BF, name="ctxT_sb", tag="ctxT_sb")
    rcp_sb = singles.tile([128, HPC * QC], FP, name="rcp_sb", tag="rcp_sb")
    v_sb = []
    for j in range(TT):
        vt = singles.tile([128, HPC, DH + 1], BF, name=f"v_sb{j}", tag=f"v_sb{j}")
        v_sb.append(vt)

    # ones columns for the fused softmax denominators (Pool engine, SBUF)
    for j in range(TT):
        nc.gpsimd.memset(v_sb[j][:, :, DH : DH + 1], 1.0)

    # ---------------- input DMAs (SP/HWDGE queue) ----------------
    xtr = xt.rearrange("p (c t) -> p c t", c=KC)
    nc.sync.dma_start(out=wk_sb[:, 0, :], in_=wk[:, 0:768])
    nc.sync.dma_start(out=xT_sb[:, 0, 0:512], in_=xtr[:, 0, 0:512])
    nc.sync.dma_start(out=wq_sb[:, 0, :], in_=wq[:, 0:768])
    for c in range(1, KC):
        nc.sync.dma_start(out=xT_sb[:, c, 0:512], in_=xtr[:, c, 0:512])
    for c in range(KC):
        nc.sync.dma_start(out=xT_sb[:, c, 512:1024], in_=xtr[:, c, 512:1024])
    nc.sync.dma_start(out=wk_sb[:, 1:3, :], in_=wk[:, 768:2304])
    nc.sync.dma_start(out=wq_sb[:, 1:3, :], in_=wq[:, 768:2304])
    nc.sync.dma_start(out=wv_sb, in_=wv.rearrange("p (c n) -> p c n", c=KC))
    nc.sync.dma_start(out=wo_sb, in_=wo.rearrange("p (m d) -> p m d", m=MC))
    nc.sync.dma_start(out=id_sb, in_=ident)

    # ---------------- atoms ----------------
    expS = {}

    def qk_half(m, n, w_sb, dst, act_copy):
        ps = wps.tile([128, 512], FP, name="ps_qk", tag="w")
        for c in range(KC):
            nc.tensor.matmul(
                ps,
                lhsT=w_sb[:, m, c * 128 : (c + 1) * 128],
                rhs=xT_sb[:, c, n * 512 : (n + 1) * 512],
                start=(c == 0),
                stop=(c == KC - 1),
            )
        if act_copy:
            nc.scalar.copy(dst[:, m, n * 512 : (n + 1) * 512], ps)
        else:
            nc.vector.tensor_copy(dst[:, m, n * 512 : (n + 1) * 512], ps)

    def score(h, qq, jq):
        # S.T for key chunks j = 4jq..4jq+3, query quarter qq, one head
        m, po = h // 2, 64 * (h % 2)
        ps = sps.tile([128, 1024], FP, name="ps_s", tag="s")
        for r in range(4):
            j = 4 * jq + r
            nc.tensor.matmul(
                ps[:, r * 256 : (r + 1) * 256],
                lhsT=kT_sb[po : po + 64, m, j * 128 : (j + 1) * 128],
                rhs=qT_sb[po : po + 64, m, qq * 256 : (qq + 1) * 256],
                start=True,
                stop=True,
            )
        ex = expp.tile([128, 1024], BF, name="ex", tag="ex")
        nc.scalar.activation(ex, ps, AF.Exp)
        expS[(h, qq, jq)] = ex

    def v_mt(mt):
        ps = wps.tile([128, DPC], FP, name="ps_v", tag="w")
        for c in range(KC):
            nc.tensor.matmul(
                ps,
                lhsT=xT_sb[:, c, mt * 128 : (mt + 1) * 128],
                rhs=wv_sb[:, c, :],
                start=(c == 0),
                stop=(c == KC - 1),
            )
        nc.vector.tensor_copy(v_sb[mt][:, :, 0:DH], ps)

    def ctx_pair(pair, qc):
        # ctx[q, dh|denom] for heads 2p,2p+1 in one psum tile [128, 130]
        pc = wps.tile([128, 130], FP, name="pc", tag="w")
        for hi in range(2):
            h = 2 * pair + hi
            col = hi * 65
            for j in range(TT):
                ex = expS[(h, qc // 2, j // 4)]
                off = (j % 4) * 256 + (qc % 2) * 128
                nc.tensor.matmul(
                    pc[:, col : col + 65],
                    lhsT=ex[:, off : off + 128],
                    rhs=v_sb[j][:, h, :],
                    start=(j == 0),
                    stop=(j == TT - 1),
                )
        for hi in range(2):
            h = 2 * pair + hi
            k = h * QC + qc
            nc.vector.reciprocal(
                rcp_sb[:, k : k + 1], pc[:, hi * 65 + 64 : hi * 65 + 65]
            )
            nc.vector.tensor_scalar_mul(
                ctxn_sb[:, qc, h * 64 : (h + 1) * 64],
                pc[:, hi * 65 : hi * 65 + 64],
                rcp_sb[:, k : k + 1],
            )

    def tpose_dma(pair, qc):
        # pairs 0/1: DMA-XBAR sbuf->sbuf bf16 transpose, off the PE/DVE path
        nc.sync.dma_start_transpose(
            out=ctxT_sb[:, pair, qc * 128 : (qc + 1) * 128],
            in_=ctxn_sb[:, qc, pair * 128 : (pair + 1) * 128],
        )

    def tpose_pe(pair, qc):
        tp = wps.tile([128, 128], BF, name="tp", tag="w")
        nc.tensor.matmul(
            tp,
            lhsT=ctxn_sb[:, qc, pair * 128 : (pair + 1) * 128],
            rhs=id_sb,
            is_transpose=True,
        )
        nc.vector.tensor_copy(ctxT_sb[:, pair, qc * 128 : (qc + 1) * 128], tp)

    def out_full(qc, tail=False):
        osb = osbp.tile([128, D], FP, name="osb1", tag="osb1")
        for n2 in range(2):
            ps = wps.tile([128, 384], FP, name="ps_o", tag="w")
            for m in range(MC):
                nc.tensor.matmul(
                    ps,
                    lhsT=ctxT_sb[:, m, qc * 128 : (qc + 1) * 128],
                    rhs=wo_sb[:, m, n2 * 384 : (n2 + 1) * 384],
                    start=(m == 0),
                    stop=(m == MC - 1),
                )
            if tail and n2 == 1:
                nc.scalar.copy(osb[:, n2 * 384 : (n2 + 1) * 384], ps)
            else:
                nc.vector.tensor_copy(osb[:, n2 * 384 : (n2 + 1) * 384], ps)
        nc.sync.dma_start(out=o1d[qc * 128 : (qc + 1) * 128, :], in_=osb)

    # ---------------- schedule ----------------
    # The ScalarE exp chain is the clock: exps run query-quarter-major
    # (for qq: for h:) so complete output columns finish every ~12.5us and
    # only the last quarter's ctx/transpose/out sits on the tail.
    # qk m0 chase first (k/q n0 with ScalarE copies) so head-0 scores
    # start while the n1 token-halves are still in flight.
    qk_half(0, 0, wk_sb, kT_sb, act_copy=True)
    qk_half(0, 0, wq_sb, qT_sb, act_copy=True)
    score(0, 0, 0)
    qk_half(0, 1, wk_sb, kT_sb, act_copy=False)
    score(0, 0, 1)
    score(1, 0, 0)
    qk_half(0, 1, wq_sb, qT_sb, act_copy=False)
    score(1, 0, 1)
    for n in range(2):
        qk_half(1, n, wk_sb, kT_sb, act_copy=False)
        qk_half(1, n, wq_sb, qT_sb, act_copy=False)
    score(2, 0, 0)
    score(2, 0, 1)
    for n in range(2):
        qk_half(2, n, wk_sb, kT_sb, act_copy=False)
        qk_half(2, n, wq_sb, qT_sb, act_copy=False)
    score(3, 0, 0)
    score(3, 0, 1)
    score(4, 0, 0)
    score(4, 0, 1)
    score(5, 0, 0)
    score(5, 0, 1)
    for mt in range(TT):
        v_mt(mt)

    for qq in range(4):
        nxt, tail = qq + 1, qq == 3
        qa, qb = 2 * qq, 2 * qq + 1
        tpose = tpose_pe if tail else tpose_dma
        for pair in range(2):
            for qc in (qa, qb):
                ctx_pair(pair, qc)
                tpose(pair, qc)
        if not tail:
            score(0, nxt, 0)
            score(0, nxt, 1)
        for qc in (qa, qb):
            ctx_pair(2, qc)
            tpose(2, qc)
        if not tail:
            score(1, nxt, 0)
            score(1, nxt, 1)
        out_full(qa, tail=tail)
        if not tail:
            score(2, nxt, 0)
            score(2, nxt, 1)
        out_full(qb, tail=tail)
        if not tail:
            for h in range(3, 6):
                score(h, nxt, 0)
                score(h, nxt, 1)


_PROGRAM = None


def build_program():
    global _PROGRAM
    if _PROGRAM is not None:
        return _PROGRAM
    nc = bacc.Bacc("TRN2", target_bir_lowering=False, debug=False, num_devices=NCORES)
    xt = nc.dram_tensor("xt", (128, KC * T), BF, kind="ExternalInput").ap()
    wk = nc.dram_tensor("wk", (128, MC * 768), BF, kind="ExternalInput").ap()
    wq = nc.dram_tensor("wq", (128, MC * 768), BF, kind="ExternalInput").ap()
    wv = nc.dram_tensor("wv", (128, KC * DPC), BF, kind="ExternalInput").ap()
    wo = nc.dram_tensor("wo", (128, MC * 768), BF, kind="ExternalInput").ap()
    ident = nc.dram_tensor("ident", (128, 128), BF, kind="ExternalInput").ap()
    out1 = nc.dram_tensor("out1", (T, D), FP, kind="ExternalOutput").ap()
    from contextlib import ExitStack

    with TileContext(nc) as tc, ExitStack() as st:
        emit_mha(tc, xt, wk, wq, wv, wo, ident, out1, st)
    nc.compile()
    _PROGRAM = nc
    return nc


def _pack_kq(w):
    # [768 d_in, 384 dout] -> [128 p, (m, c, 128)] with d_in = c*128+p
    return np.ascontiguousarray(
        w.reshape(KC, 128, MC, 128).transpose(1, 2, 0, 3).reshape(128, MC * 768)
    ).astype(BF_NP)


def make_in_maps(x, Wq, Wk, Wv, Wo):
    x = np.asarray(x, dtype=np.float32)
    ident = np.eye(128, dtype=np.float32).astype(BF_NP)
    in_maps = []
    xTs = []
    for b in range(B):
        xb = x[b].T  # [768, 1024]
        xTs.append(
            np.ascontiguousarray(
                xb.reshape(KC, 128, T).transpose(1, 0, 2).reshape(128, KC * T)
            ).astype(BF_NP)
        )
    for core in range(NCORES):
        b, hh = core // 2, core % 2
        sl = slice(hh * DPC, (hh + 1) * DPC)
        wvT = np.asarray(Wv)[sl].T.astype(np.float32)  # [768, 384]
        woT = np.asarray(Wo)[:, sl].T.astype(np.float32)  # [384, 768]
        in_maps.append(
            {
                "xt": xTs[b],
                "wq": _pack_kq((np.asarray(Wq)[sl] * 0.125).T.astype(np.float32)),
                "wk": _pack_kq(np.asarray(Wk)[sl].T.astype(np.float32)),
                "wv": np.ascontiguousarray(
                    wvT.reshape(KC, 128, DPC).transpose(1, 0, 2).reshape(128, KC * DPC)
                ).astype(BF_NP),
                "wo": np.ascontiguousarray(
                    woT.reshape(MC, 128, 768).transpose(1, 0, 2).reshape(128, MC * 768)
                ).astype(BF_NP),
                "ident": ident,
            }
        )
    return in_maps


def kernel(x, Wq, Wk, Wv, Wo, bo):
    nc = build_program()
    in_maps = make_in_maps(x, Wq, Wk, Wv, Wo)
    res = run_bass_kernel_spmd(nc, in_maps, core_ids=list(range(NCORES)))
    bo = np.asarray(bo, dtype=np.float32)
    out = np.empty((B, T, D), dtype=np.float32)
    for b in range(B):
        out[b] = res.results[2 * b]["out1"] + res.results[2 * b + 1]["out1"] + bo
    return out


# revision 9
# speedup vs baseline: 1.0349x; 1.0082x over previous
"""MultiHeadAttention Trainium2 kernel (8 NeuronCores, SPMD).

Reference computation (B=4, T=1024, D=768, H=12, Dh=64):
    q = x @ Wq.T ; k = x @ Wk.T ; v = x @ Wv.T       (per-head reshape)
    attn = softmax((q @ k.T) / 8)
    out = (attn @ v) @ Wo.T + bo

Sharding: 8 cores = 4 batches x 2 head-halves (6 heads each). Each core
emits two [1024, 768] partials (head-chunks m0+m1 and m2) of the output
projection; the host sums the four partials per batch and adds the bias.

All device data is bf16 (host-converted, fp32 PSUM accumulation), which
runs matmuls at 1 cycle/row for any moving-dim size and halves DMA bytes.

Per-core dataflow:
    qT,kT = (W x)  in [dh(384), t] layout (per m-chunk of 128 = 2 heads)
    v     = (x Wv) in [t, 6*(64+1)] tiles; col 64 of each head block is
            ones so the ctx matmul also emits softmax denominators
    S.T psum [kt(4x128), q(256)] = kT_head.T @ qT_head  (K=64, N=256)
    expS  = exp(S.T) bf16 via ScalarE reading PSUM; quarter-width q
            tiles so the last head's attention staggers per 2 q-chunks
    ctx[q, 65] psum += expS_j.T @ [v_j | 1]       (K=kt chunks, N=65)
        col 64 = denominator; normalize with DVE reciprocal +
        per-partition tensor_scalar_mul (denominator is per-q = per-row)
    ctxT: head-pairs 0,1 via DMA-XBAR sbuf transpose (bf16), pair 2 via
        PE transpose (latency-critical tail)
    out1[q, :] = ctxT_0.T @ Wo_0 + ctxT_1.T @ Wo_1   (psum accum, mid)
    out2[q, :] = ctxT_2.T @ Wo_2                     (tail, no add)
"""

import numpy as np
import ml_dtypes

import concourse.mybir as mybir
from concourse import bacc
from concourse.tile import TileContext
from concourse.bass_utils import run_bass_kernel_spmd

FP = mybir.dt.float32
BF = mybir.dt.bfloat16
AF = mybir.ActivationFunctionType
BF_NP = ml_dtypes.bfloat16

B, T, D = 4, 1024, 768
H, DH = 12, 64
NCORES = 8
HPC = 6           # heads per core
DPC = HPC * DH    # 384 head-dims per core
KC = D // 128     # 6 contraction chunks of d_in
MC = DPC // 128   # 3 chunks of per-core head dims (2 heads each)
QC = T // 128     # 8 query chunks
TT = T // 128     # 8 key chunks


def emit_mha(tc, xt, wk, wq, wv, wo, ident, o1d, ctx):
    nc = tc.nc

    singles = ctx.enter_context(tc.tile_pool(name="singles", bufs=1))
    # scores psum: [128,1024] fp32 = 2 banks each; 2 bufs = 4 banks
    sps = ctx.enter_context(tc.tile_pool(name="sps", bufs=2, space="PSUM"))
    # shared work psum (qk/v/ctx/transpose/out): 4 bufs x 1 bank = 4 banks
    wps = ctx.enter_context(tc.tile_pool(name="wps", bufs=4, space="PSUM"))
    expp = ctx.enter_context(tc.tile_pool(name="expp", bufs=34))
    osbp = ctx.enter_context(tc.tile_pool(name="osbp", bufs=4))

    # ---------------- SBUF singles ----------------
    xT_sb = singles.tile([128, KC, T], BF, name="xT_sb", tag="xT_sb")
    wk_sb = singles.tile([128, MC, 768], BF, name="wk_sb", tag="wk_sb")
    wq_sb = singles.tile([128, MC, 768], BF, name="wq_sb", tag="wq_sb")
    wv_sb = singles.tile([128, KC, DPC], BF, name="wv_sb", tag="wv_sb")
    wo_sb = singles.tile([128, MC, 768], BF, name="wo_sb", tag="wo_sb")
    id_sb = singles.tile([128, 128], BF, name="id_sb", tag="id_sb")
    kT_sb = singles.tile([128, MC, T], BF, name="kT_sb", tag="kT_sb")
    qT_sb = singles.tile([128, MC, T], BF, name="qT_sb", tag="qT_sb")
    ctxn_sb = singles.tile([128, QC, DPC], BF, name="ctxn_sb", tag="ctxn_sb")
    ctxT_sb = singles.tile([128, MC, T], BF, name="ctxT_sb", tag="ctxT_sb")
    rcp_sb = singles.tile([128, HPC * QC], FP, name="rcp_sb", tag="rcp_sb")
    v_sb = []
    for j in range(TT):
        vt = singles.tile([128, HPC, DH + 1], BF, name=f"v_sb{j}", tag=f"v_sb{j}")
        v_sb.append(vt)

    # ones columns for the fused softmax denominators (Pool engine, SBUF)
    for j in range(TT):
        nc.gpsimd.memset(v_sb[j][:, :, DH : DH + 1], 1.0)

    # ---------------- input DMAs (SP/HWDGE queue) ----------------
    xtr = xt.rearrange("p (c t) -> p c t", c=KC)
    nc.sync.dma_start(out=wk_sb[:, 0, :], in_=wk[:, 0:768])
    nc.sync.dma_start(out=xT_sb[:, 0, 0:512], in_=xtr[:, 0, 0:512])
    nc.sync.dma_start(out=wq_sb[:, 0, :], in_=wq[:, 0:768])
    for c in range(1, KC):
        nc.sync.dma_start(out=xT_sb[:, c, 0:512], in_=xtr[:, c, 0:512])
    nc.sync.dma_start(out=wv_sb, in_=wv.rearrange("p (c n) -> p c n", c=KC))
    for c in range(KC):
        nc.sync.dma_start(out=xT_sb[:, c, 512:1024], in_=xtr[:, c, 512:1024])
    nc.sync.dma_start(out=wk_sb[:, 1:3, :], in_=wk[:, 768:2304])
    nc.sync.dma_start(out=wq_sb[:, 1:3, :], in_=wq[:, 768:2304])
    nc.sync.dma_start(out=wo_sb, in_=wo.rearrange("p (m d) -> p m d", m=MC))
    nc.sync.dma_start(out=id_sb, in_=ident)

    # ---------------- atoms ----------------
    expS = {}

    def qk_half(m, n, w_sb, dst, act_copy):
        ps = wps.tile([128, 512], FP, name="ps_qk", tag="w")
        for c in range(KC):
            nc.tensor.matmul(
                ps,
                lhsT=w_sb[:, m, c * 128 : (c + 1) * 128],
                rhs=xT_sb[:, c, n * 512 : (n + 1) * 512],
                start=(c == 0),
                stop=(c == KC - 1),
            )
        if act_copy:
            nc.scalar.copy(dst[:, m, n * 512 : (n + 1) * 512], ps)
        else:
            nc.vector.tensor_copy(dst[:, m, n * 512 : (n + 1) * 512], ps)

    def score(h, qq, jq):
        # S.T for key chunks j = 4jq..4jq+3, query quarter qq, one head
        m, po = h // 2, 64 * (h % 2)
        ps = sps.tile([128, 1024], FP, name="ps_s", tag="s")
        for r in range(4):
            j = 4 * jq + r
            nc.tensor.matmul(
                ps[:, r * 256 : (r + 1) * 256],
                lhsT=kT_sb[po : po + 64, m, j * 128 : (j + 1) * 128],
                rhs=qT_sb[po : po + 64, m, qq * 256 : (qq + 1) * 256],
                start=True,
                stop=True,
            )
        ex = expp.tile([128, 1024], BF, name="ex", tag="ex")
        nc.scalar.activation(ex, ps, AF.Exp)
        expS[(h, qq, jq)] = ex

    def v_mt(mt):
        ps = wps.tile([128, DPC], FP, name="ps_v", tag="w")
        for c in range(KC):
            nc.tensor.matmul(
                ps,
                lhsT=xT_sb[:, c, mt * 128 : (mt + 1) * 128],
                rhs=wv_sb[:, c, :],
                start=(c == 0),
                stop=(c == KC - 1),
            )
        nc.vector.tensor_copy(v_sb[mt][:, :, 0:DH], ps)

    def ctx_pair(pair, qc):
        # ctx[q, dh|denom] for heads 2p,2p+1 in one psum tile [128, 130]
        pc = wps.tile([128, 130], FP, name="pc", tag="w")
        for hi in range(2):
            h = 2 * pair + hi
            col = hi * 65
            for j in range(TT):
                ex = expS[(h, qc // 2, j // 4)]
                off = (j % 4) * 256 + (qc % 2) * 128
                nc.tensor.matmul(
                    pc[:, col : col + 65],
                    lhsT=ex[:, off : off + 128],
                    rhs=v_sb[j][:, h, :],
                    start=(j == 0),
                    stop=(j == TT - 1),
                )
        for hi in range(2):
            h = 2 * pair + hi
            k = h * QC + qc
            nc.vector.reciprocal(
                rcp_sb[:, k : k + 1], pc[:, hi * 65 + 64 : hi * 65 + 65]
            )
            nc.vector.tensor_scalar_mul(
                ctxn_sb[:, qc, h * 64 : (h + 1) * 64],
                pc[:, hi * 65 : hi * 65 + 64],
                rcp_sb[:, k : k + 1],
            )

    def tpose_dma(pair, qc):
        # pairs 0/1: DMA-XBAR sbuf->sbuf bf16 transpose, off the PE/DVE path
        nc.sync.dma_start_transpose(
            out=ctxT_sb[:, pair, qc * 128 : (qc + 1) * 128],
            in_=ctxn_sb[:, qc, pair * 128 : (pair + 1) * 128],
        )

    def tpose_pe(pair, qc):
        tp = wps.tile([128, 128], BF, name="tp", tag="w")
        nc.tensor.matmul(
            tp,
            lhsT=ctxn_sb[:, qc, pair * 128 : (pair + 1) * 128],
            rhs=id_sb,
            is_transpose=True,
        )
        nc.vector.tensor_copy(ctxT_sb[:, pair, qc * 128 : (qc + 1) * 128], tp)

    def out_full(qc, tail=False):
        osb = osbp.tile([128, D], FP, name="osb1", tag="osb1")
        for n2 in range(2):
            ps = wps.tile([128, 384], FP, name="ps_o", tag="w")
            for m in range(MC):
                nc.tensor.matmul(
                    ps,
                    lhsT=ctxT_sb[:, m, qc * 128 : (qc + 1) * 128],
                    rhs=wo_sb[:, m, n2 * 384 : (n2 + 1) * 384],
                    start=(m == 0),
                    stop=(m == MC - 1),
                )
            if tail and n2 == 1:
                nc.scalar.copy(osb[:, n2 * 384 : (n2 + 1) * 384], ps)
            else:
                nc.vector.tensor_copy(osb[:, n2 * 384 : (n2 + 1) * 384], ps)
            if tail:
                nc.sync.dma_start(
                    out=o1d[qc * 128 : (qc + 1) * 128, n2 * 384 : (n2 + 1) * 384],
                    in_=osb[:, n2 * 384 : (n2 + 1) * 384],
                )
        if not tail:
            nc.sync.dma_start(out=o1d[qc * 128 : (qc + 1) * 128, :], in_=osb)

    # ---------------- schedule ----------------
    # The ScalarE exp chain is the clock: exps run query-quarter-major
    # (for qq: for h:) so complete output columns finish every ~12.5us and
    # only the last quarter's ctx/transpose/out sits on the tail. Emission
    # order per engine approximates readiness order; scores for the next
    # quarter interleave with this quarter's ctx/out so PE follows the
    # sps-slot rotation without starving the exp chain.
    qk_half(0, 0, wk_sb, kT_sb, act_copy=True)
    qk_half(0, 0, wq_sb, qT_sb, act_copy=True)
    score(0, 0, 0)
    v_mt(0)
    v_mt(1)
    qk_half(0, 1, wk_sb, kT_sb, act_copy=False)
    score(0, 0, 1)
    score(1, 0, 0)
    v_mt(2)
    v_mt(3)
    qk_half(0, 1, wq_sb, qT_sb, act_copy=False)
    score(1, 0, 1)
    for n in range(2):
        qk_half(1, n, wk_sb, kT_sb, act_copy=False)
        qk_half(1, n, wq_sb, qT_sb, act_copy=False)
    score(2, 0, 0)
    score(2, 0, 1)
    for n in range(2):
        qk_half(2, n, wk_sb, kT_sb, act_copy=False)
        qk_half(2, n, wq_sb, qT_sb, act_copy=False)
    score(3, 0, 0)
    score(3, 0, 1)
    v_mt(4)
    v_mt(5)
    score(4, 0, 0)
    score(4, 0, 1)
    v_mt(6)
    v_mt(7)
    score(5, 0, 0)
    score(5, 0, 1)

    for qq in range(4):
        nxt, tail = qq + 1, qq == 3
        qa, qb = 2 * qq, 2 * qq + 1
        for qc in (qa, qb):
            ctx_pair(0, qc)
            tpose_dma(0, qc)
        for qc in (qa, qb):
            ctx_pair(1, qc)
            tpose_dma(1, qc)
        if not tail:
            score(0, nxt, 0)
            score(0, nxt, 1)
        for qc in (qa, qb):
            ctx_pair(2, qc)
            (tpose_pe if tail else tpose_dma)(2, qc)
        if not tail:
            score(1, nxt, 0)
            score(1, nxt, 1)
        out_full(qa, tail=tail)
        if not tail:
            score(2, nxt, 0)
            score(2, nxt, 1)
        out_full(qb, tail=tail)
        if not tail:
            for h in range(3, 6):
                score(h, nxt, 0)
                score(h, nxt, 1)


_PROGRAM = None


def build_program():
    global _PROGRAM
    if _PROGRAM is not None:
        return _PROGRAM
    nc = bacc.Bacc("TRN2", target_bir_lowering=False, debug=False, num_devices=NCORES)
    xt = nc.dram_tensor("xt", (128, KC * T), BF, kind="ExternalInput").ap()
    wk = nc.dram_tensor("wk", (128, MC * 768), BF, kind="ExternalInput").ap()
    wq = nc.dram_tensor("wq", (128, MC * 768), BF, kind="ExternalInput").ap()
    wv = nc.dram_tensor("wv", (128, KC * DPC), BF, kind="ExternalInput").ap()
    wo = nc.dram_tensor("wo", (128, MC * 768), BF, kind="ExternalInput").ap()
    ident = nc.dram_tensor("ident", (128, 128), BF, kind="ExternalInput").ap()
    out1 = nc.dram_tensor("out1", (T, D), FP, kind="ExternalOutput").ap()
    from contextlib import ExitStack

    with TileContext(nc) as tc, ExitStack() as st:
        emit_mha(tc, xt, wk, wq, wv, wo, ident, out1, st)
    nc.compile()
    _PROGRAM = nc
    return nc


def _pack_kq(w):
    # [768 d_in, 384 dout] -> [128 p, (m, c, 128)] with d_in = c*128+p
    return np.ascontiguousarray(
        w.reshape(KC, 128, MC, 128).transpose(1, 2, 0, 3).reshape(128, MC * 768)
    ).astype(BF_NP)


def make_in_maps(x, Wq, Wk, Wv, Wo):
    x = np.asarray(x, dtype=np.float32)
    ident = np.eye(128, dtype=np.float32).astype(BF_NP)
    in_maps = []
    xTs = []
    for b in range(B):
        xb = x[b].T  # [768, 1024]
        xTs.append(
            np.ascontiguousarray(
                xb.reshape(KC, 128, T).transpose(1, 0, 2).reshape(128, KC * T)
            ).astype(BF_NP)
        )
    for core in range(NCORES):
        b, hh = core // 2, core % 2
        sl = slice(hh * DPC, (hh + 1) * DPC)
        wvT = np.asarray(Wv)[sl].T.astype(np.float32)  # [768, 384]
        woT = np.asarray(Wo)[:, sl].T.astype(np.float32)  # [384, 768]
        in_maps.append(
            {
                "xt": xTs[b],
                "wq": _pack_kq((np.asarray(Wq)[sl] * 0.125).T.astype(np.float32)),
                "wk": _pack_kq(np.asarray(Wk)[sl].T.astype(np.float32)),
                "wv": np.ascontiguousarray(
                    wvT.reshape(KC, 128, DPC).transpose(1, 0, 2).reshape(128, KC * DPC)
                ).astype(BF_NP),
                "wo": np.ascontiguousarray(
                    woT.reshape(MC, 128, 768).transpose(1, 0, 2).reshape(128, MC * 768)
                ).astype(BF_NP),
                "ident": ident,
            }
        )
    return in_maps


def kernel(x, Wq, Wk, Wv, Wo, bo):
    nc = build_program()
    in_maps = make_in_maps(x, Wq, Wk, Wv, Wo)
    res = run_bass_kernel_spmd(nc, in_maps, core_ids=list(range(NCORES)))
    bo = np.asarray(bo, dtype=np.float32)
    out = np.empty((B, T, D), dtype=np.float32)
    for b in range(B):
        out[b] = res.results[2 * b]["out1"] + res.results[2 * b + 1]["out1"] + bo
    return out
